# revision 1
# baseline (speedup 1.0000x reference)
"""MoE block (top-2 routed 3x3 conv experts) Trainium2 Bass kernel.

Strategy: data-parallel over batch, 2 samples per core on 8 cores.
Since the conv is linear in the kernel, combine the top-2 expert kernels
with the routing probabilities first (w_comb = sum_e p_e * W_e), then do a
single 3x3 SAME conv per sample, plus bias + residual.

Conv-as-matmul layout: x is stored zero-padded [130x130] per channel in
SBUF, flat, with partitions 0-63 = channels and partitions 64-127 = the
same channels shifted by +2 pixels. A single [128, 4x128] rhs read then
provides taps (dy,-1) on top and (dy,+1) on the bottom half, so the six
dx=+-1 taps are three K=128 matmuls; the three dx=0 taps are K=128
matmuls with zero weights on the bottom half (K=64 matmuls measure ~1.8x
slower). All conv matmuls run in float32r (single-pass fp32 mode,
~235ns per [128,64]x[128,512] MM).

DMA: each engine's DGE lane moves ~150-190 GB/s and its transfers
serialize, so: SP lane = top halves + expert weights A + even out
tiles; ACT lane = bottom halves (rows 64-127 first, feeding pooled) +
odd out tiles; gpsimd SWDGE = border memsets first, then tiny gate
weights + expert weights B. Conv tiles dep on the whole XX tile, so
lanes carry as little as possible before the last x chunk. Pooled GAP
partials: top chunks 0,1 via ACT ACTIVATE+accum (interleaves with its
DMA queue), bottom chunks 2,3 on DVE; the gate matmul uses a
[wg1; wg1]-stacked lhsT to fold the cross-half sum. Gate math runs on
DVE except the softmax exp (ACT, ordered before the late ACT DMAs).
"""
import numpy as np
from contextlib import ExitStack

import concourse.bass as bass
import concourse.tile as tile
from concourse import bacc, mybir
from concourse.bass_utils import run_bass_kernel_spmd
from concourse.tile import add_dep_helper

F32 = mybir.dt.float32
F32R = mybir.dt.float32r
AX = mybir.AxisListType
OP = mybir.AluOpType
ACTF = mybir.ActivationFunctionType

B, C, H, W, E, GH = 16, 64, 128, 128, 8, 16
NCORES = 8
SPB = B // NCORES          # samples per core
HP, WP = H + 2, W + 2      # 130
FLAT = HP * WP             # 16900
NT = H // 4                # 32 conv tiles of 4 rows each
RB = 32                    # x-load chunk rows
GATE_SPLIT = 18            # emit sample-1 gate work after this many s0 conv tiles

_cache = {}


def _emit_borders(nc, XX):
    """Zero the padded borders (disjoint from the DMA-written interiors)."""
    nc.gpsimd.memset(XX[0:64, 0:130].bitcast(F32), 0.0)
    mid_top = XX[0:64, 130:16770].rearrange("p (r c) -> p r c", c=WP)
    nc.gpsimd.memset(mid_top[:, :, 0:1].bitcast(F32), 0.0)
    nc.gpsimd.memset(mid_top[:, :, 129:130].bitcast(F32), 0.0)
    nc.gpsimd.memset(XX[0:64, 16770:16900].bitcast(F32), 0.0)
    nc.gpsimd.memset(XX[64:128, 0:129].bitcast(F32), 0.0)
    mid_bot = XX[64:128, 257:16767].rearrange("p (r c) -> p r c", c=WP)
    nc.gpsimd.memset(mid_bot[:, :, 0:2].bitcast(F32), 0.0)
    nc.gpsimd.memset(XX[64:128, 16767:16900].bitcast(F32), 0.0)


def _emit_sample_loads(nc, pools, s, XX, xs_ap, mid_sp=None):
    """Input DMAs + pooled partial sums for sample s.

    SP lane (slower DGE): top chunks 0,1,2.
    ACT lane (faster DGE): bottom chunks 2,3 (rows 64-127, pooled
    inputs), bottom chunks 0,1, top chunk 3, then [mid_sp()].
    DVE sums top chunks 0,1 (partitions 0-63) and bottom chunks 2,3
    (partitions 64-127) as they land.
    """
    f = pools
    top_int = XX[0:64, 131:16771].rearrange("p (r c) -> p r c", c=WP)
    bot_int = XX[64:128, 129:16769].rearrange("p (r c) -> p r c", c=WP)

    def top(k, eng):
        eng.dma_start(
            top_int[:, RB * k : RB * (k + 1), 0:128],
            xs_ap[s, :, RB * k : RB * (k + 1), :],
        )

    def bot(k):
        return nc.scalar.dma_start(
            bot_int[:, RB * k : RB * (k + 1), 0:128],
            xs_ap[s, :, RB * k : RB * (k + 1), :],
        )

    part = f["gate"].tile([128, 2], F32, tag="part", name=f"part{s}")

    top(0, nc.sync)
    top(1, nc.sync)
    if mid_sp is not None:
        mid_sp()
    top(2, nc.sync)
    top(3, nc.sync)
    bot(2)
    bot(3)
    # ACT-lane compute: pooled partials of top chunks 0,1
    scrA = f["scratch"].tile([64, RB, 128], F32, tag="scrA", name=f"scrA{s}")
    for k in (0, 1):
        nc.scalar.activation(
            scrA[:],
            top_int[:, RB * k : RB * (k + 1), 0:128].bitcast(F32),
            ACTF.Copy,
            accum_out=part[0:64, k : k + 1],
        )
    bot_dmas = [bot(0), bot(1)]
    # DVE: pooled partials of bottom chunks 2,3 (partitions 64-127)
    def dve_part(view, dst):
        scrB = f["scratch"].tile([128, RB, 128], F32, tag="scrB", name=f"scrB{s}_{dst[0]}_{dst[1]}")
        nc.vector.tensor_scalar(
            scrB[dst[0] : dst[0] + 64],
            view.bitcast(F32),
            0.0,
            0.0,
            OP.add,
            OP.add,
            accum_out=part[dst[0] : dst[0] + 64, dst[1] : dst[1] + 1],
        )

    dve_part(bot_int[:, RB * 2 : RB * 3, 0:128], (64, 0))
    dve_part(bot_int[:, RB * 3 : RB * 4, 0:128], (64, 1))
    pooled = f["gate"].tile([128, 1], F32, tag="pooled", name=f"pooled{s}")
    nc.vector.tensor_reduce(pooled, part[:], axis=AX.X, op=OP.add)
    return pooled, bot_dmas


def _emit_sample_gate(nc, pools, s, pooled, consts):
    """Gate MLP + softmax + top-2 + combined weights/bias for one sample.

    Uses exp-without-max-sub (logits are small) and folds the top-2 mask
    and renormalization:  w8 = (u>=m2)*u / (sum((u>=m2)*u) + sum(u)*1e-8)
    which equals the reference's normalized-probs formula exactly.
    Returns (wcombr, b_comb, exp_inst).
    """
    f = pools
    g = f["gate"]
    wg1x2_sb, bg1_sb, wg2_sb, bg2_sb, bexp_sb, wpsA_sb, wpsB_sb, ones = consts
    n = lambda base: f"{base}{s}"

    h_ps = f["gpsum"].tile([GH, 1], F32, tag="gps", name=n("h_ps"))
    nc.tensor.matmul(h_ps[:], lhsT=wg1x2_sb[:], rhs=pooled[:], start=True, stop=True)
    # h_ext = [relu(pooled_sum @ (wg1/(H*W)) + bg1); 1.0] — the trailing 1.0
    # row turns bg2 into a wg2 row in the next matmul
    h_ext = g.tile([GH + 1, 1], F32, tag="h_ext", name=n("h_ext"))
    nc.gpsimd.dma_start(h_ext[GH : GH + 1, 0:1], ones[0:1, 0:1])
    nc.vector.tensor_scalar(
        h_ext[0:GH, :], h_ps[:], bg1_sb[:], 0.0, OP.add, OP.max
    )

    lg_ps = f["gpsum"].tile([1, E], F32, tag="gps", name=n("lg_ps"))
    nc.tensor.matmul(lg_ps[:], lhsT=h_ext[:], rhs=wg2_sb[:], start=True, stop=True)

    # u = exp(logits) (unnormalized softmax; |logits| is tiny, no max-sub)
    u = g.tile([1, E], F32, tag="u", name=n("u"))
    exp_inst = nc.scalar.activation(u[:], lg_ps[:], ACTF.Exp)
    usum = g.tile([1, 1], F32, tag="usum", name=n("usum"))
    nc.vector.tensor_reduce(usum[:], u[:], axis=AX.X, op=OP.add)
    # top-2: pm = (u < max)*u (valid since u>0), m2 = 2nd max, spv = (u>=m2)*u
    m1p = g.tile([1, 1], F32, tag="m1p", name=n("m1p"))
    nc.vector.tensor_reduce(m1p[:], u[:], axis=AX.X, op=OP.max)
    pm = g.tile([1, E], F32, tag="pm", name=n("pm"))
    nc.vector.scalar_tensor_tensor(pm[:], u[:], m1p[:], u[:], op0=OP.is_lt, op1=OP.mult)
    m2 = g.tile([1, 1], F32, tag="m2", name=n("m2"))
    nc.vector.tensor_reduce(m2[:], pm[:], axis=AX.X, op=OP.max)
    spv = g.tile([1, E], F32, tag="spv", name=n("spv"))
    nc.vector.scalar_tensor_tensor(spv[:], u[:], m2[:], u[:], op0=OP.is_ge, op1=OP.mult)
    dsum = g.tile([1, 1], F32, tag="dsum", name=n("dsum"))
    nc.vector.tensor_reduce(dsum[:], spv[:], axis=AX.X, op=OP.add)
    dd = g.tile([1, 1], F32, tag="dd", name=n("dd"))
    nc.vector.scalar_tensor_tensor(dd[:], usum[:], 1e-8, dsum[:], op0=OP.mult, op1=OP.add)
    rr = g.tile([1, 1], F32, tag="rr", name=n("rr"))
    nc.vector.reciprocal(rr[:], dd[:])
    w8 = g.tile([1, E], F32, tag="w8", name=n("w8"))
    nc.vector.tensor_scalar_mul(w8[:], spv[:], rr[:])

    # broadcast w8 down all 128 partitions: [128, E] = ones[1,128]^T @ w8[1,E]
    wb_ps = f["gpsum"].tile([128, E], F32, tag="wbps", name=n("wb_ps"), bufs=1)
    nc.tensor.matmul(wb_ps[:], lhsT=ones[:], rhs=w8[:], start=True, stop=True)
    wb128 = wb_ps

    # combined bias path (off critical path): b_comb = b_exp^T @ w8^T
    w8c_ps = f["gpsum"].tile([E, 1], F32, tag="gps", name=n("w8c_ps"))
    nc.tensor.matmul(w8c_ps[:], lhsT=w8[:], rhs=ones[:, 0:1], start=True, stop=True)
    w8col = g.tile([E, 1], F32, tag="w8col", name=n("w8col"))
    nc.vector.tensor_copy(w8col[:], w8c_ps[:])
    bc_ps = f["gpsum"].tile([C, 1], F32, tag="gps", name=n("bc_ps"))
    nc.tensor.matmul(bc_ps[:], lhsT=bexp_sb[:], rhs=w8col[:], start=True, stop=True)
    b_comb = g.tile([C, 1], F32, tag="b_comb", name=n("b_comb"))
    nc.vector.tensor_copy(b_comb[:], bc_ps[:])

    # combined conv weights: one fused MAC chain over [128, 6, C]
    # (slots 0-2 = paired dx taps, 3-5 = dx=0 taps w/ zero bottom rows)
    wcomb = f["wcomb"].tile([128, 6, C], F32, tag="wcomb", name=n("wcomb"))
    nc.vector.tensor_scalar_mul(wcomb[:], wpsA_sb[:, 0], wb128[:, 0:1])
    for e in range(1, E):
        src_w = wpsA_sb[:, e] if e < 4 else wpsB_sb[:, e - 4]
        nc.vector.scalar_tensor_tensor(
            wcomb[:], src_w, wb128[:, e : e + 1], wcomb[:],
            op0=OP.mult, op1=OP.add,
        )
    wcombr = f["wcomb"].tile([128, 6, C], F32R, tag="wcombr", name=n("wcombr"))
    nc.vector.tensor_copy(wcombr[:], wcomb[:])
    return wcombr, b_comb, exp_inst


def _emit_conv_tiles(nc, pools, s, XX, wcombr, b_comb, out_ap, t_range):
    """Conv tiles (4 output rows each) for sample s."""
    f = pools
    XX3 = XX[:, 0:FLAT].rearrange("p (r c) -> p r c", c=WP)
    for t in t_range:
        ps = f["cpsum"].tile([C, 4 * W], F32, tag="cps", name=f"cps{s}_{t}")
        r0 = 4 * t
        for dyi in range(3):
            nc.tensor.matmul(
                ps[:],
                lhsT=wcombr[:, dyi, :],
                rhs=XX3[:, r0 + dyi : r0 + dyi + 4, 0:128],
                start=(dyi == 0),
                stop=False,
            )
        for dyi in range(3):
            nc.tensor.matmul(
                ps[:],
                lhsT=wcombr[:, 3 + dyi, :],
                rhs=XX3[:, r0 + dyi : r0 + dyi + 4, 1:129],
                start=False,
                stop=(dyi == 2),
            )
        out_sb = f["stage"].tile([C, 4, W], F32, tag="stage", name=f"ost{s}_{t}")
        nc.vector.scalar_tensor_tensor(
            out_sb[:],
            ps[:].rearrange("p (a b) -> p a b", b=W),
            b_comb[:],
            XX3[0:64, r0 + 1 : r0 + 5, 1:129].bitcast(F32),
            op0=OP.add,
            op1=OP.add,
        )
        eng = nc.sync if t % 2 == 0 else nc.scalar
        eng.dma_start(out_ap[s, :, r0 : r0 + 4, :], out_sb[:])


def build_program():
    if "nc" in _cache:
        return _cache["nc"]
    nc = bacc.Bacc("TRN2", target_bir_lowering=False, debug=False, enable_asserts=False)
    xs_ap = nc.dram_tensor("xs", [SPB, C, H, W], F32R, kind="ExternalInput").ap()
    wpsA_d = nc.dram_tensor("wpsA", [128, E // 2, 6, C], F32, kind="ExternalInput").ap()
    wpsB_d = nc.dram_tensor("wpsB", [128, E // 2, 6, C], F32, kind="ExternalInput").ap()
    wg1_d = nc.dram_tensor("wg1", [128, GH], F32, kind="ExternalInput").ap()
    bg1_d = nc.dram_tensor("bg1", [GH, 1], F32, kind="ExternalInput").ap()
    wg2_d = nc.dram_tensor("wg2", [GH + 1, E], F32, kind="ExternalInput").ap()
    bg2_d = nc.dram_tensor("bg2", [1, E], F32, kind="ExternalInput").ap()
    bexp_d = nc.dram_tensor("b_exp", [E, C], F32, kind="ExternalInput").ap()
    out_ap = nc.dram_tensor("out", [SPB, C, H, W], F32, kind="ExternalOutput").ap()

    with tile.TileContext(nc) as tc, ExitStack() as ctx:
        pools = {
            "const": ctx.enter_context(tc.tile_pool(name="const", bufs=1)),
            "xx": ctx.enter_context(tc.tile_pool(name="xx", bufs=SPB)),
            "gate": ctx.enter_context(tc.tile_pool(name="gate", bufs=2)),
            "wcomb": ctx.enter_context(tc.tile_pool(name="wcomb", bufs=2)),
            "stage": ctx.enter_context(tc.tile_pool(name="stage", bufs=6)),
            "scratch": ctx.enter_context(tc.tile_pool(name="scratch", bufs=1)),
            "gpsum": ctx.enter_context(tc.tile_pool(name="gpsum", bufs=1, space="PSUM")),
            "cpsum": ctx.enter_context(tc.tile_pool(name="cpsum", bufs=6, space="PSUM")),
        }
        cp = pools["const"]
        # XX tiles + their border memsets first on gpsimd, so the memsets
        # never delay the x-chunk DMAs whose regions they border
        XX0 = pools["xx"].tile([128, FLAT], F32R, tag="XX", name="XX0")
        XX1 = pools["xx"].tile([128, FLAT], F32R, tag="XX", name="XX1")
        _emit_borders(nc, XX0)
        _emit_borders(nc, XX1)
        ones = cp.tile([1, 128], F32)
        nc.gpsimd.memset(ones[:], 1.0)
        # prewarm the ACT exp table before the ACT lane fills with DMAs
        warm = cp.tile([1, 1], F32)
        nc.scalar.activation(warm[:], ones[:, 0:1], ACTF.Exp)
        # tiny gate weights + expert weights B on the gpsimd SWDGE lane
        wg1x2_sb = cp.tile([128, GH], F32)
        nc.gpsimd.dma_start(wg1x2_sb[:], wg1_d[:])
        bg1_sb = cp.tile([GH, 1], F32)
        nc.gpsimd.dma_start(bg1_sb[:], bg1_d[:])
        wg2_sb = cp.tile([GH + 1, E], F32)
        nc.gpsimd.dma_start(wg2_sb[:], wg2_d[:])
        bg2_sb = cp.tile([1, E], F32)
        nc.gpsimd.dma_start(bg2_sb[:], bg2_d[:])
        bexp_sb = cp.tile([E, C], F32)
        nc.gpsimd.dma_start(bexp_sb[:], bexp_d[:])
        wpsA_sb = cp.tile([128, E // 2, 6, C], F32)
        wpsB_sb = cp.tile([128, E // 2, 6, C], F32)
        nc.gpsimd.dma_start(wpsB_sb[:], wpsB_d[:])

        def load_wpsA():
            nc.sync.dma_start(wpsA_sb[:], wpsA_d[:])

        pooled0, bots0 = _emit_sample_loads(nc, pools, 0, XX0, xs_ap, mid_sp=load_wpsA)
        consts = (wg1x2_sb, bg1_sb, wg2_sb, bg2_sb, bexp_sb, wpsA_sb, wpsB_sb, ones)

        g0 = _emit_sample_gate(nc, pools, 0, pooled0, consts)
        add_dep_helper(bots0[0].ins, g0[2].ins, sync=False,
                       reason="s0 late bottom DMAs after s0 softmax exp")
        pooled1, bots1 = _emit_sample_loads(nc, pools, 1, XX1, xs_ap)

        _emit_conv_tiles(nc, pools, 0, XX0, *g0[:2], out_ap, range(0, GATE_SPLIT))
        g1 = _emit_sample_gate(nc, pools, 1, pooled1, consts)
        _emit_conv_tiles(nc, pools, 0, XX0, *g0[:2], out_ap, range(GATE_SPLIT, NT))
        _emit_conv_tiles(nc, pools, 1, XX1, *g1[:2], out_ap, range(0, NT))

    nc.compile()
    _cache["nc"] = nc
    return nc


def _round_fp32r(a):
    """Round fp32 array to the fp32r grid (RNE to 11-bit mantissa, low 12
    bits of the fp32 word zeroed) — what the PE consumes in fp32r mode."""
    u = np.ascontiguousarray(a, dtype=np.float32).view(np.uint32)
    r = (u + np.uint32(0x7FF) + ((u >> np.uint32(12)) & np.uint32(1))) & np.uint32(
        0xFFFFF000
    )
    return r.view(np.float32)


def host_prep(x, wg1, bg1, wg2, bg2, w_exp, b_exp):
    """Host-side layout prep + per-core sharding. Returns in_maps list."""
    x = _round_fp32r(np.asarray(x, dtype=np.float32))
    wg1 = np.asarray(wg1, dtype=np.float32)
    bg1 = np.asarray(bg1, dtype=np.float32).reshape(GH, 1)
    wg2 = np.asarray(wg2, dtype=np.float32)
    bg2 = np.asarray(bg2, dtype=np.float32).reshape(1, E)
    w_exp = np.asarray(w_exp, dtype=np.float32)
    b_exp = np.asarray(b_exp, dtype=np.float32)

    # w_exp [E, O, I, KH, KW] -> wt [I, E, KH, KW, O]
    wt = np.transpose(w_exp, (2, 0, 3, 4, 1))
    # paired taps: top partitions = dx=-1, bottom = dx=+1
    wpair = np.concatenate([wt[:, :, :, 0, :], wt[:, :, :, 2, :]], axis=0)
    # single taps: dx=0 on top, zeros on bottom
    wsing = np.concatenate([wt[:, :, :, 1, :], np.zeros_like(wt[:, :, :, 1, :])], axis=0)
    # merged [128, E, 6, O]: slots 0-2 pairs, 3-5 singles
    wps = np.concatenate([wpair, wsing], axis=2)

    shared = {
        "wpsA": np.ascontiguousarray(wps[:, 0:4]),
        "wpsB": np.ascontiguousarray(wps[:, 4:8]),
        "wg1": np.ascontiguousarray(np.concatenate([wg1, wg1], axis=0) / (H * W)),
        "bg1": np.ascontiguousarray(bg1),
        "wg2": np.ascontiguousarray(np.concatenate([wg2, bg2], axis=0)),
        "bg2": np.ascontiguousarray(bg2),
        "b_exp": np.ascontiguousarray(b_exp),
    }
    return [
        {"xs": np.ascontiguousarray(x[SPB * k : SPB * (k + 1)]), **shared}
        for k in range(NCORES)
    ]


def kernel(x, wg1, bg1, wg2, bg2, w_exp, b_exp):
    nc = build_program()
    in_maps = host_prep(x, wg1, bg1, wg2, bg2, w_exp, b_exp)
    res = run_bass_kernel_spmd(nc, in_maps, list(range(NCORES)))
    return np.concatenate([res.results[k]["out"] for k in range(NCORES)], axis=0)



# revision 3
# speedup vs baseline: 1.0898x; 1.0898x over previous
"""MoE block (top-2 routed 3x3 conv experts) Trainium2 Bass kernel.

Strategy: data-parallel over batch, 2 samples per core on 8 cores.
The conv is linear in the kernel, so the top-2 expert kernels are
combined with the routing probabilities first (w_comb = sum_e p_e W_e),
then one 3x3 SAME conv per sample runs as matmuls.

v2 changes over the baseline:
- fp16 everywhere on the conv path (x, expert weights, output staging):
  halves HBM traffic, doubles DVE/ACT elementwise rates, enables FWL
  weight loads. Gate math stays fp32 (top-2 margins are ~1e-5 in prob).
- Residual folded into the center-tap expert weights on host
  (W_e[center] += I; valid since the routing probs sum to 1), so the
  post-conv op is a pure bias add.
- Column-tiled conv: tiles t (PSUM partitions 0:64, PE col group 0) and
  t+1 (partitions 64:128, col group 1) run their 6-matmul chains
  concurrently in the two halves of the 128-wide PE array. One
  [128, 512] bias-add per pair (alternating ACT/DVE) replaces two
  [64, 512] ops.

Conv-as-matmul layout: x is stored zero-padded [130x130] per channel in
SBUF, flat, with partitions 0-63 = channels and partitions 64-127 = the
same channels shifted by +2 pixels, so a [128, 4x128] rhs read provides
taps (dy,-1) on top and (dy,+1) on the bottom half; dx=0 taps are
K=128 matmuls with zero weights on the bottom half.

DMA queues (sync/scalar HWDGE + gpsimd SWDGE share ~358 GB/s HBM):
SP = top halves + expert weights A + out pairs; ACT = bottom halves
(rows 64-127 first, feeding pooled) + out pairs; gpsimd = border
memsets, gate weights, expert weights B, out pairs.
"""
import numpy as np
from contextlib import ExitStack

import concourse.bass as bass
import concourse.tile as tile
from concourse import bacc, mybir
from concourse.bass_utils import run_bass_kernel_spmd
from concourse.tile import add_dep_helper

F32 = mybir.dt.float32
F16 = mybir.dt.float16
AX = mybir.AxisListType
OP = mybir.AluOpType
ACTF = mybir.ActivationFunctionType

B, C, H, W, E, GH = 16, 64, 128, 128, 8, 16
NCORES = 8
SPB = B // NCORES          # samples per core
HP, WP = H + 2, W + 2      # 130
FLAT = HP * WP             # 16900
NP = H // 8                # 16 conv pairs (8 output rows each)
RB = 32                    # x-load chunk rows
GATE_SPLIT = 9             # emit sample-1 gate work after this many s0 conv pairs

_cache = {}


def _emit_borders(nc, XX):
    """Zero the padded borders (disjoint from the DMA-written interiors)."""
    nc.gpsimd.memset(XX[0:64, 0:130], 0.0)
    mid_top = XX[0:64, 130:16770].rearrange("p (r c) -> p r c", c=WP)
    nc.gpsimd.memset(mid_top[:, :, 0:1], 0.0)
    nc.gpsimd.memset(mid_top[:, :, 129:130], 0.0)
    nc.gpsimd.memset(XX[0:64, 16770:16900], 0.0)
    nc.gpsimd.memset(XX[64:128, 0:129], 0.0)
    mid_bot = XX[64:128, 257:16767].rearrange("p (r c) -> p r c", c=WP)
    nc.gpsimd.memset(mid_bot[:, :, 0:2], 0.0)
    nc.gpsimd.memset(XX[64:128, 16767:16900], 0.0)


def _emit_sample_loads(nc, pools, s, XX, xs_ap, mid_sp=None):
    """Input DMAs + pooled partial sums for sample s.

    SP lane: top chunks 0-3 (+ mid_sp between 1 and 2).
    ACT lane: bottom chunks 2,3 (rows 64-127, pooled inputs), then 0,1.
    ACT engine sums top chunks 0,1 (partitions 0-63); DVE sums bottom
    chunks 2,3 (partitions 64-127) as they land.
    """
    f = pools
    top_int = XX[0:64, 131:16771].rearrange("p (r c) -> p r c", c=WP)
    bot_int = XX[64:128, 129:16769].rearrange("p (r c) -> p r c", c=WP)

    def top(k):
        nc.sync.dma_start(
            top_int[:, RB * k : RB * (k + 1), 0:128],
            xs_ap[s, :, RB * k : RB * (k + 1), :],
        )

    def bot(k):
        return nc.scalar.dma_start(
            bot_int[:, RB * k : RB * (k + 1), 0:128],
            xs_ap[s, :, RB * k : RB * (k + 1), :],
        )

    part = f["gate"].tile([128, 2], F32, tag="part", name=f"part{s}")

    top(0)
    top(1)
    if mid_sp is not None:
        mid_sp()
    top(2)
    top(3)
    bot(2)
    bot(3)
    # ACT-lane compute: pooled partials of top chunks 0,1
    scrA = f["scratch"].tile([64, RB, 128], F16, tag="scrA", name=f"scrA{s}")
    for k in (0, 1):
        nc.scalar.activation(
            scrA[:],
            top_int[:, RB * k : RB * (k + 1), 0:128],
            ACTF.Copy,
            accum_out=part[0:64, k : k + 1],
        )
    bot_dmas = [bot(0), bot(1)]
    # DVE: pooled partials of bottom chunks 2,3 (partitions 64-127)
    def dve_part(view, dst):
        scrB = f["scratch"].tile([128, RB, 128], F16, tag="scrB", name=f"scrB{s}_{dst[0]}_{dst[1]}")
        nc.vector.tensor_scalar(
            scrB[dst[0] : dst[0] + 64],
            view,
            0.0,
            0.0,
            OP.add,
            OP.add,
            accum_out=part[dst[0] : dst[0] + 64, dst[1] : dst[1] + 1],
        )

    dve_part(bot_int[:, RB * 2 : RB * 3, 0:128], (64, 0))
    dve_part(bot_int[:, RB * 3 : RB * 4, 0:128], (64, 1))
    pooled = f["gate"].tile([128, 1], F32, tag="pooled", name=f"pooled{s}")
    nc.vector.tensor_reduce(pooled, part[:], axis=AX.X, op=OP.add)
    return pooled, bot_dmas


def _emit_sample_gate(nc, pools, s, pooled, consts):
    """Gate MLP + softmax + top-2 + combined weights/bias for one sample.

    Uses exp-without-max-sub (logits are small) and folds the top-2 mask
    and renormalization:  w8 = (u>=m2)*u / (sum((u>=m2)*u) + sum(u)*1e-8)
    which equals the reference's normalized-probs formula exactly.
    Returns (wcombr, b_comb128, exp_inst).
    """
    f = pools
    g = f["gate"]
    wg1x2_sb, bg1_sb, wg2_sb, bg2_sb, bexp_sb, wps_sb, ones = consts
    n = lambda base: f"{base}{s}"

    h_ps = f["gpsum"].tile([GH, 1], F32, tag="gps", name=n("h_ps"))
    nc.tensor.matmul(h_ps[:], lhsT=wg1x2_sb[:], rhs=pooled[:], start=True, stop=True)
    # h_ext = [relu(pooled_sum @ (wg1/(H*W)) + bg1); 1.0] — the trailing 1.0
    # row turns bg2 into a wg2 row in the next matmul
    h_ext = g.tile([GH + 1, 1], F32, tag="h_ext", name=n("h_ext"))
    nc.gpsimd.dma_start(h_ext[GH : GH + 1, 0:1], ones[0:1, 0:1])
    nc.vector.tensor_scalar(
        h_ext[0:GH, :], h_ps[:], bg1_sb[:], 0.0, OP.add, OP.max
    )

    lg_ps = f["gpsum"].tile([1, E], F32, tag="gps", name=n("lg_ps"))
    nc.tensor.matmul(lg_ps[:], lhsT=h_ext[:], rhs=wg2_sb[:], start=True, stop=True)

    # u = exp(logits) (unnormalized softmax; |logits| is tiny, no max-sub)
    u = g.tile([1, E], F32, tag="u", name=n("u"))
    exp_inst = nc.scalar.activation(u[:], lg_ps[:], ACTF.Exp)
    usum = g.tile([1, 1], F32, tag="usum", name=n("usum"))
    nc.vector.tensor_reduce(usum[:], u[:], axis=AX.X, op=OP.add)
    # top-2: pm = (u < max)*u (valid since u>0), m2 = 2nd max, spv = (u>=m2)*u
    m1p = g.tile([1, 1], F32, tag="m1p", name=n("m1p"))
    nc.vector.tensor_reduce(m1p[:], u[:], axis=AX.X, op=OP.max)
    pm = g.tile([1, E], F32, tag="pm", name=n("pm"))
    nc.vector.scalar_tensor_tensor(pm[:], u[:], m1p[:], u[:], op0=OP.is_lt, op1=OP.mult)
    m2 = g.tile([1, 1], F32, tag="m2", name=n("m2"))
    nc.vector.tensor_reduce(m2[:], pm[:], axis=AX.X, op=OP.max)
    spv = g.tile([1, E], F32, tag="spv", name=n("spv"))
    nc.vector.scalar_tensor_tensor(spv[:], u[:], m2[:], u[:], op0=OP.is_ge, op1=OP.mult)
    dsum = g.tile([1, 1], F32, tag="dsum", name=n("dsum"))
    nc.vector.tensor_reduce(dsum[:], spv[:], axis=AX.X, op=OP.add)
    dd = g.tile([1, 1], F32, tag="dd", name=n("dd"))
    nc.vector.scalar_tensor_tensor(dd[:], usum[:], 1e-8, dsum[:], op0=OP.mult, op1=OP.add)
    rr = g.tile([1, 1], F32, tag="rr", name=n("rr"))
    nc.vector.reciprocal(rr[:], dd[:])
    w8 = g.tile([1, E], F32, tag="w8", name=n("w8"))
    nc.vector.tensor_scalar_mul(w8[:], spv[:], rr[:])

    # broadcast w8 down all 128 partitions: [128, E] = ones[1,128]^T @ w8[1,E]
    wb_ps = f["gpsum"].tile([128, E], F32, tag="wbps", name=n("wb_ps"), bufs=1)
    nc.tensor.matmul(wb_ps[:], lhsT=ones[:], rhs=w8[:], start=True, stop=True)
    wb128 = wb_ps

    # combined bias path (off critical path): b_comb128 = (b_exp
    # duplicated over both partition halves)^T @ w8^T
    w8c_ps = f["gpsum"].tile([E, 1], F32, tag="gps", name=n("w8c_ps"))
    nc.tensor.matmul(w8c_ps[:], lhsT=w8[:], rhs=ones[:, 0:1], start=True, stop=True)
    w8col = g.tile([E, 1], F32, tag="w8col", name=n("w8col"))
    nc.vector.tensor_copy(w8col[:], w8c_ps[:])
    bc_ps = f["gpsum"].tile([128, 1], F32, tag="gps2", name=n("bc_ps"))
    nc.tensor.matmul(bc_ps[:], lhsT=bexp_sb[:], rhs=w8col[:], start=True, stop=True)
    b_comb = g.tile([128, 1], F32, tag="b_comb", name=n("b_comb"))
    nc.vector.tensor_copy(b_comb[:], bc_ps[:])

    # combined conv weights: one fused MAC chain over [128, 6, C]
    # (slots 0-2 = paired dx taps, 3-5 = dx=0 taps w/ zero bottom rows;
    # slot 4 top half carries +I for the residual shortcut)
    wcomb = f["wcomb"].tile([128, 6, C], F32, tag="wcomb", name=n("wcomb"))
    nc.vector.tensor_scalar_mul(wcomb[:], wps_sb[:, 0], wb128[:, 0:1])
    for e in range(1, E):
        nc.vector.scalar_tensor_tensor(
            wcomb[:], wps_sb[:, e], wb128[:, e : e + 1], wcomb[:],
            op0=OP.mult, op1=OP.add,
        )
    wcombr = f["wcomb"].tile([128, 6, C], F16, tag="wcombr", name=n("wcombr"))
    nc.vector.tensor_copy(wcombr[:], wcomb[:])
    return wcombr, b_comb, exp_inst


def _emit_conv_pairs(nc, pools, s, XX, wcombr, b_comb, out_ap, p_range):
    """Conv pairs (2 tiles x 4 output rows) for sample s.

    Tile A (rows 8p..8p+4) accumulates in PSUM partitions 0:64 via PE
    column group 0; tile B (rows 8p+4..8p+8) in partitions 64:128 via
    column group 1. The two 6-matmul chains run concurrently in the two
    halves of the PE array.
    """
    f = pools
    XX3 = XX[:, 0:FLAT].rearrange("p (r c) -> p r c", c=WP)
    for p in p_range:
        rA, rB = 8 * p, 8 * p + 4
        ps = f["cpsum"].tile([128, 4, W], F32, tag="cps", name=f"cps{s}_{p}")
        for dyi in range(3):
            for half, r0 in ((0, rA), (64, rB)):
                nc.tensor.matmul(
                    ps[half : half + 64],
                    lhsT=wcombr[:, dyi, :],
                    rhs=XX3[:, r0 + dyi : r0 + dyi + 4, 0:128],
                    start=(dyi == 0),
                    stop=False,
                )
        for dyi in range(3):
            for half, r0 in ((0, rA), (64, rB)):
                nc.tensor.matmul(
                    ps[half : half + 64],
                    lhsT=wcombr[:, 3 + dyi, :],
                    rhs=XX3[:, r0 + dyi : r0 + dyi + 4, 1:129],
                    start=False,
                    stop=(dyi == 2),
                )
        out_sb = f["stage"].tile([128, 4, W], F16, tag="stage", name=f"ost{s}_{p}")
        if p % 2 == 0:
            nc.scalar.activation(out_sb[:], ps[:], ACTF.Identity, bias=b_comb[:, 0:1])
        else:
            nc.vector.tensor_scalar_add(out_sb[:], ps[:], b_comb[:, 0:1])
        eng = (nc.sync, nc.scalar, nc.gpsimd)[p % 3]
        eng.dma_start(out_ap[s, :, rA : rA + 4, :], out_sb[0:64])
        eng.dma_start(out_ap[s, :, rB : rB + 4, :], out_sb[64:128])


def build_program():
    if "nc" in _cache:
        return _cache["nc"]
    nc = bacc.Bacc("TRN2", target_bir_lowering=False, debug=False, enable_asserts=False)
    xs_ap = nc.dram_tensor("xs", [SPB, C, H, W], F16, kind="ExternalInput").ap()
    wpsA_d = nc.dram_tensor("wpsA", [128, E // 2, 6, C], F16, kind="ExternalInput").ap()
    wpsB_d = nc.dram_tensor("wpsB", [128, E // 2, 6, C], F16, kind="ExternalInput").ap()
    wg1_d = nc.dram_tensor("wg1", [128, GH], F32, kind="ExternalInput").ap()
    bg1_d = nc.dram_tensor("bg1", [GH, 1], F32, kind="ExternalInput").ap()
    wg2_d = nc.dram_tensor("wg2", [GH + 1, E], F32, kind="ExternalInput").ap()
    bg2_d = nc.dram_tensor("bg2", [1, E], F32, kind="ExternalInput").ap()
    bexp_d = nc.dram_tensor("b_exp", [E, 128], F32, kind="ExternalInput").ap()
    out_ap = nc.dram_tensor("out", [SPB, C, H, W], F16, kind="ExternalOutput").ap()

    with tile.TileContext(nc) as tc, ExitStack() as ctx:
        pools = {
            "const": ctx.enter_context(tc.tile_pool(name="const", bufs=1)),
            "xx": ctx.enter_context(tc.tile_pool(name="xx", bufs=SPB)),
            "gate": ctx.enter_context(tc.tile_pool(name="gate", bufs=2)),
            "wcomb": ctx.enter_context(tc.tile_pool(name="wcomb", bufs=2)),
            "stage": ctx.enter_context(tc.tile_pool(name="stage", bufs=6)),
            "scratch": ctx.enter_context(tc.tile_pool(name="scratch", bufs=1)),
            "gpsum": ctx.enter_context(tc.tile_pool(name="gpsum", bufs=1, space="PSUM")),
            "cpsum": ctx.enter_context(tc.tile_pool(name="cpsum", bufs=5, space="PSUM")),
        }
        cp = pools["const"]
        # XX tiles + their border memsets first on gpsimd, so the memsets
        # never delay the x-chunk DMAs whose regions they border
        XX0 = pools["xx"].tile([128, FLAT], F16, tag="XX", name="XX0")
        XX1 = pools["xx"].tile([128, FLAT], F16, tag="XX", name="XX1")
        _emit_borders(nc, XX0)
        _emit_borders(nc, XX1)
        ones = cp.tile([1, 128], F32)
        nc.gpsimd.memset(ones[:], 1.0)
        # prewarm the ACT exp table before the ACT lane fills with DMAs
        warm = cp.tile([1, 1], F32)
        nc.scalar.activation(warm[:], ones[:, 0:1], ACTF.Exp)
        # tiny gate weights + expert weights B on the gpsimd SWDGE lane
        wg1x2_sb = cp.tile([128, GH], F32)
        nc.gpsimd.dma_start(wg1x2_sb[:], wg1_d[:])
        bg1_sb = cp.tile([GH, 1], F32)
        nc.gpsimd.dma_start(bg1_sb[:], bg1_d[:])
        wg2_sb = cp.tile([GH + 1, E], F32)
        nc.gpsimd.dma_start(wg2_sb[:], wg2_d[:])
        bg2_sb = cp.tile([1, E], F32)
        nc.gpsimd.dma_start(bg2_sb[:], bg2_d[:])
        bexp_sb = cp.tile([E, 128], F32)
        nc.gpsimd.dma_start(bexp_sb[:], bexp_d[:])
        wps_sb = cp.tile([128, E, 6, C], F16)
        nc.gpsimd.dma_start(wps_sb[:, E // 2 :], wpsB_d[:])

        def load_wpsA():
            nc.sync.dma_start(wps_sb[:, : E // 2], wpsA_d[:])

        pooled0, bots0 = _emit_sample_loads(nc, pools, 0, XX0, xs_ap, mid_sp=load_wpsA)
        consts = (wg1x2_sb, bg1_sb, wg2_sb, bg2_sb, bexp_sb, wps_sb, ones)

        g0 = _emit_sample_gate(nc, pools, 0, pooled0, consts)
        add_dep_helper(bots0[0].ins, g0[2].ins, sync=False,
                       reason="s0 late bottom DMAs after s0 softmax exp")
        pooled1, bots1 = _emit_sample_loads(nc, pools, 1, XX1, xs_ap)

        _emit_conv_pairs(nc, pools, 0, XX0, *g0[:2], out_ap, range(0, GATE_SPLIT))
        g1 = _emit_sample_gate(nc, pools, 1, pooled1, consts)
        _emit_conv_pairs(nc, pools, 0, XX0, *g0[:2], out_ap, range(GATE_SPLIT, NP))
        _emit_conv_pairs(nc, pools, 1, XX1, *g1[:2], out_ap, range(0, NP))

    nc.compile()
    _cache["nc"] = nc
    return nc


def host_prep(x, wg1, bg1, wg2, bg2, w_exp, b_exp):
    """Host-side layout prep + per-core sharding. Returns in_maps list."""
    x = np.asarray(x, dtype=np.float32).astype(np.float16)
    wg1 = np.asarray(wg1, dtype=np.float32)
    bg1 = np.asarray(bg1, dtype=np.float32).reshape(GH, 1)
    wg2 = np.asarray(wg2, dtype=np.float32)
    bg2 = np.asarray(bg2, dtype=np.float32).reshape(1, E)
    w_exp = np.asarray(w_exp, dtype=np.float32)
    b_exp = np.asarray(b_exp, dtype=np.float32)

    # w_exp [E, O, I, KH, KW] -> wt [I, E, KH, KW, O]
    wt = np.transpose(w_exp, (2, 0, 3, 4, 1)).copy()
    # residual shortcut: out += x == each expert's center tap += I
    # (routing probs sum to 1 up to the reference's 1e-8 epsilon)
    wt[:, :, 1, 1, :] += np.eye(C, dtype=np.float32)[:, None, :]
    # paired taps: top partitions = dx=-1, bottom = dx=+1
    wpair = np.concatenate([wt[:, :, :, 0, :], wt[:, :, :, 2, :]], axis=0)
    # single taps: dx=0 on top, zeros on bottom
    wsing = np.concatenate([wt[:, :, :, 1, :], np.zeros_like(wt[:, :, :, 1, :])], axis=0)
    # merged [128, E, 6, O]: slots 0-2 pairs, 3-5 singles
    wps = np.concatenate([wpair, wsing], axis=2).astype(np.float16)

    shared = {
        "wpsA": np.ascontiguousarray(wps[:, 0:4]),
        "wpsB": np.ascontiguousarray(wps[:, 4:8]),
        "wg1": np.ascontiguousarray(np.concatenate([wg1, wg1], axis=0) / (H * W)),
        "bg1": np.ascontiguousarray(bg1),
        "wg2": np.ascontiguousarray(np.concatenate([wg2, bg2], axis=0)),
        "bg2": np.ascontiguousarray(bg2),
        "b_exp": np.ascontiguousarray(np.concatenate([b_exp, b_exp], axis=1)),
    }
    return [
        {"xs": np.ascontiguousarray(x[SPB * k : SPB * (k + 1)]), **shared}
        for k in range(NCORES)
    ]


def kernel(x, wg1, bg1, wg2, bg2, w_exp, b_exp):
    nc = build_program()
    in_maps = host_prep(x, wg1, bg1, wg2, bg2, w_exp, b_exp)
    res = run_bass_kernel_spmd(nc, in_maps, list(range(NCORES)))
    return np.concatenate(
        [res.results[k]["out"].astype(np.float32) for k in range(NCORES)], axis=0
    )


# revision 4
# speedup vs baseline: 1.6412x; 1.5060x over previous
"""MoE block (top-2 routed 3x3 conv experts) Trainium2 Bass kernel.

Strategy: data-parallel over batch, 2 samples per core on 8 cores.
The conv is linear in the kernel, so the top-2 expert kernels are
combined with the routing probabilities first (w_comb = sum_e p_e W_e),
then one 3x3 SAME conv per sample runs as matmuls.

v3 design:
- fp16 on the conv path (x, expert weights, output staging): halves HBM
  traffic and enables FWL weight loads. Gate math stays fp32 (top-2
  margins are ~1e-5 in prob).
- x is padded to [C, 130*130] on the HOST, so every input DMA writes
  one fully contiguous flat range per partition (the v2 per-row 256B
  segments sat below the 512B SDMA line-rate threshold and ran at
  ~50 GB/s). No device-side border memsets needed.
- Partitions 0-63 = padded channels; partitions 64-127 = same data
  shifted +2 elements (second contiguous DMA of the same HBM tensor),
  so a [128, 4x128] rhs read provides taps (dy,-1) on top and (dy,+1)
  on the bottom half; dx=0 taps use zero weights on the bottom half.
- Residual folded into the center-tap expert weights on host
  (W_e[center] += I; valid since the routing probs sum to 1), so the
  post-conv op is a pure bias add.
- Column-tiled conv: tiles t (PSUM partitions 0:64, PE col group 0) and
  t+1 (partitions 64:128, col group 1) run their 6-matmul chains
  concurrently in the two halves of the 128-wide PE array. One
  [128, 512] bias-add per pair (alternating ACT/DVE).

DMA queues (sync/scalar HWDGE + gpsimd SWDGE share ~358 GB/s HBM):
SP = top copies + expert weights A + half the out pairs; ACT = bottom
(+2-shifted) copies, chunks 2,3 first to feed the DVE pooled partials;
gpsimd = gate weights + expert weights B.
"""
import numpy as np
from contextlib import ExitStack

import concourse.bass as bass
import concourse.tile as tile
from concourse import bacc, mybir
from concourse.bass_utils import run_bass_kernel_spmd
from concourse.tile import add_dep_helper

F32 = mybir.dt.float32
F16 = mybir.dt.float16
AX = mybir.AxisListType
OP = mybir.AluOpType
ACTF = mybir.ActivationFunctionType

B, C, H, W, E, GH = 16, 64, 128, 128, 8, 16
NCORES = 8
SPB = B // NCORES          # samples per core
HP, WP = H + 2, W + 2      # 130
FLAT = HP * WP             # 16900
NP = H // 8                # 16 conv pairs (8 output rows each)
QS = FLAT // 4             # 4225: flat quarter-chunk
BEND = FLAT - 2            # last valid element of the shifted bottom copy
GATE_SPLIT = 9             # emit sample-1 gate work after this many s0 conv pairs

_cache = {}


def _emit_sample_loads(nc, pools, s, XX, xs_ap, mid_sp=None):
    """Input DMAs + pooled partial sums for sample s.

    Top copy (partitions 0:64) = xpad quarters on SP; bottom copy
    (partitions 64:128, shifted +2) on ACT, quarters 2,3 first so the
    DVE pooled partials can start early. ACT engine sums top quarters
    0,1; DVE sums xpad[2*QS:FLAT) via the bottom copy. Exact coverage:
    top(0,1) = xpad[0:2*QS), bottom dst [2*QS-2:BEND) = xpad[2*QS:FLAT).
    """
    f = pools

    def top(k):
        nc.sync.dma_start(
            XX[0:64, QS * k : QS * (k + 1)],
            xs_ap[s, :, QS * k : QS * (k + 1)],
        )

    def bot(k):
        a = max(QS * k - 2, 0)
        b = min(QS * (k + 1) - 2, BEND)
        return nc.scalar.dma_start(XX[64:128, a:b], xs_ap[s, :, a + 2 : b + 2])

    part = f["gate"].tile([128, 2], F32, tag="part", name=f"part{s}")

    top(0)
    top(1)
    if mid_sp is not None:
        mid_sp()
    top(2)
    top(3)
    bot(2)
    bot(3)
    # ACT-lane compute: pooled partials of top quarters 0,1
    scrA = f["scratch"].tile([64, QS], F16, tag="scrA", name=f"scrA{s}")
    for k in (0, 1):
        nc.scalar.activation(
            scrA[:],
            XX[0:64, QS * k : QS * (k + 1)],
            ACTF.Copy,
            accum_out=part[0:64, k : k + 1],
        )
    bot_dmas = [bot(0), bot(1)]
    # DVE: pooled partials of xpad[2QS:3QS) and [3QS:FLAT) via bottom copy
    def dve_part(a, b, dst):
        scrB = f["scratch"].tile([128, QS], F16, tag="scrB", name=f"scrB{s}_{dst[1]}")
        nc.vector.tensor_scalar(
            scrB[64:128, 0 : b - a],
            XX[64:128, a:b],
            0.0,
            0.0,
            OP.add,
            OP.add,
            accum_out=part[dst[0] : dst[0] + 64, dst[1] : dst[1] + 1],
        )

    dve_part(2 * QS - 2, 3 * QS - 2, (64, 0))
    dve_part(3 * QS - 2, BEND, (64, 1))
    pooled = f["gate"].tile([128, 1], F32, tag="pooled", name=f"pooled{s}")
    nc.vector.tensor_reduce(pooled, part[:], axis=AX.X, op=OP.add)
    return pooled, bot_dmas


def _emit_sample_gate(nc, pools, s, pooled, consts):
    """Gate MLP + softmax + top-2 + combined weights/bias for one sample.

    Uses exp-without-max-sub (logits are small) and folds the top-2 mask
    and renormalization:  w8 = (u>=m2)*u / (sum((u>=m2)*u) + sum(u)*1e-8)
    which equals the reference's normalized-probs formula exactly.
    Returns (wcombr, b_comb128, exp_inst).
    """
    f = pools
    g = f["gate"]
    wg1x2_sb, bg1_sb, wg2_sb, bg2_sb, bexp_sb, wps_sb, ones = consts
    n = lambda base: f"{base}{s}"

    h_ps = f["gpsum"].tile([GH, 1], F32, tag="gps", name=n("h_ps"))
    nc.tensor.matmul(h_ps[:], lhsT=wg1x2_sb[:], rhs=pooled[:], start=True, stop=True)
    # h_ext = [relu(pooled_sum @ (wg1/(H*W)) + bg1); 1.0] — the trailing 1.0
    # row turns bg2 into a wg2 row in the next matmul
    h_ext = g.tile([GH + 1, 1], F32, tag="h_ext", name=n("h_ext"))
    nc.gpsimd.dma_start(h_ext[GH : GH + 1, 0:1], ones[0:1, 0:1])
    nc.vector.tensor_scalar(
        h_ext[0:GH, :], h_ps[:], bg1_sb[:], 0.0, OP.add, OP.max
    )

    lg_ps = f["gpsum"].tile([1, E], F32, tag="gps", name=n("lg_ps"))
    nc.tensor.matmul(lg_ps[:], lhsT=h_ext[:], rhs=wg2_sb[:], start=True, stop=True)

    # u = exp(logits) (unnormalized softmax; |logits| is tiny, no max-sub)
    u = g.tile([1, E], F32, tag="u", name=n("u"))
    exp_inst = nc.scalar.activation(u[:], lg_ps[:], ACTF.Exp)
    usum = g.tile([1, 1], F32, tag="usum", name=n("usum"))
    nc.vector.tensor_reduce(usum[:], u[:], axis=AX.X, op=OP.add)
    # top-2: pm = (u < max)*u (valid since u>0), m2 = 2nd max, spv = (u>=m2)*u
    m1p = g.tile([1, 1], F32, tag="m1p", name=n("m1p"))
    nc.vector.tensor_reduce(m1p[:], u[:], axis=AX.X, op=OP.max)
    pm = g.tile([1, E], F32, tag="pm", name=n("pm"))
    nc.vector.scalar_tensor_tensor(pm[:], u[:], m1p[:], u[:], op0=OP.is_lt, op1=OP.mult)
    m2 = g.tile([1, 1], F32, tag="m2", name=n("m2"))
    nc.vector.tensor_reduce(m2[:], pm[:], axis=AX.X, op=OP.max)
    spv = g.tile([1, E], F32, tag="spv", name=n("spv"))
    nc.vector.scalar_tensor_tensor(spv[:], u[:], m2[:], u[:], op0=OP.is_ge, op1=OP.mult)
    dsum = g.tile([1, 1], F32, tag="dsum", name=n("dsum"))
    nc.vector.tensor_reduce(dsum[:], spv[:], axis=AX.X, op=OP.add)
    dd = g.tile([1, 1], F32, tag="dd", name=n("dd"))
    nc.vector.scalar_tensor_tensor(dd[:], usum[:], 1e-8, dsum[:], op0=OP.mult, op1=OP.add)
    rr = g.tile([1, 1], F32, tag="rr", name=n("rr"))
    nc.vector.reciprocal(rr[:], dd[:])
    w8 = g.tile([1, E], F32, tag="w8", name=n("w8"))
    nc.vector.tensor_scalar_mul(w8[:], spv[:], rr[:])

    # broadcast w8 down all 128 partitions: [128, E] = ones[1,128]^T @ w8[1,E]
    wb_ps = f["gpsum"].tile([128, E], F32, tag="wbps", name=n("wb_ps"), bufs=1)
    nc.tensor.matmul(wb_ps[:], lhsT=ones[:], rhs=w8[:], start=True, stop=True)
    wb128 = wb_ps

    # combined bias path (off critical path): b_comb128 = (b_exp
    # duplicated over both partition halves)^T @ w8^T
    w8c_ps = f["gpsum"].tile([E, 1], F32, tag="gps", name=n("w8c_ps"))
    nc.tensor.matmul(w8c_ps[:], lhsT=w8[:], rhs=ones[:, 0:1], start=True, stop=True)
    w8col = g.tile([E, 1], F32, tag="w8col", name=n("w8col"))
    nc.vector.tensor_copy(w8col[:], w8c_ps[:])
    bc_ps = f["gpsum"].tile([128, 1], F32, tag="gps2", name=n("bc_ps"))
    nc.tensor.matmul(bc_ps[:], lhsT=bexp_sb[:], rhs=w8col[:], start=True, stop=True)
    b_comb = g.tile([128, 1], F32, tag="b_comb", name=n("b_comb"))
    nc.vector.tensor_copy(b_comb[:], bc_ps[:])

    # combined conv weights: one fused MAC chain over [128, 6, C]
    # (slots 0-2 = paired dx taps, 3-5 = dx=0 taps w/ zero bottom rows;
    # slot 4 top half carries +I for the residual shortcut)
    wcomb = f["wcomb"].tile([128, 6, C], F32, tag="wcomb", name=n("wcomb"))
    nc.vector.tensor_scalar_mul(wcomb[:], wps_sb[:, 0], wb128[:, 0:1])
    for e in range(1, E):
        nc.vector.scalar_tensor_tensor(
            wcomb[:], wps_sb[:, e], wb128[:, e : e + 1], wcomb[:],
            op0=OP.mult, op1=OP.add,
        )
    wcombr = f["wcomb"].tile([128, 6, C], F16, tag="wcombr", name=n("wcombr"))
    nc.vector.tensor_copy(wcombr[:], wcomb[:])
    return wcombr, b_comb, exp_inst


def _emit_conv_pairs(nc, pools, s, XX, wcombr, b_comb, out_ap, p_range):
    """Conv pairs (2 tiles x 4 output rows) for sample s.

    Tile A (rows 8p..8p+4) accumulates in PSUM partitions 0:64 via PE
    column group 0; tile B (rows 8p+4..8p+8) in partitions 64:128 via
    column group 1. The two 6-matmul chains run concurrently in the two
    halves of the PE array.
    """
    f = pools
    XX3 = XX[:, 0:FLAT].rearrange("p (r c) -> p r c", c=WP)
    for p in p_range:
        rA, rB = 8 * p, 8 * p + 4
        ps = f["cpsum"].tile([128, 4, W], F32, tag="cps", name=f"cps{s}_{p}")
        for dyi in range(3):
            for half, r0 in ((0, rA), (64, rB)):
                nc.tensor.matmul(
                    ps[half : half + 64],
                    lhsT=wcombr[:, dyi, :],
                    rhs=XX3[:, r0 + dyi : r0 + dyi + 4, 0:128],
                    start=(dyi == 0),
                    stop=False,
                )
        for dyi in range(3):
            for half, r0 in ((0, rA), (64, rB)):
                nc.tensor.matmul(
                    ps[half : half + 64],
                    lhsT=wcombr[:, 3 + dyi, :],
                    rhs=XX3[:, r0 + dyi : r0 + dyi + 4, 1:129],
                    start=False,
                    stop=(dyi == 2),
                )
        out_sb = f["stage"].tile([128, 4, W], F16, tag="stage", name=f"ost{s}_{p}")
        if p % 2 == 0:
            nc.scalar.activation(out_sb[:], ps[:], ACTF.Identity, bias=b_comb[:, 0:1])
        else:
            nc.vector.tensor_scalar_add(out_sb[:], ps[:], b_comb[:, 0:1])
        eng = (nc.sync, nc.scalar)[p % 2]
        eng.dma_start(out_ap[s, :, rA : rA + 4, :], out_sb[0:64])
        eng.dma_start(out_ap[s, :, rB : rB + 4, :], out_sb[64:128])


def build_program():
    if "nc" in _cache:
        return _cache["nc"]
    nc = bacc.Bacc("TRN2", target_bir_lowering=False, debug=False, enable_asserts=False)
    xs_ap = nc.dram_tensor("xs", [SPB, C, FLAT], F16, kind="ExternalInput").ap()
    wpsA_d = nc.dram_tensor("wpsA", [128, E // 2, 6, C], F16, kind="ExternalInput").ap()
    wpsB_d = nc.dram_tensor("wpsB", [128, E // 2, 6, C], F16, kind="ExternalInput").ap()
    wg1_d = nc.dram_tensor("wg1", [128, GH], F32, kind="ExternalInput").ap()
    bg1_d = nc.dram_tensor("bg1", [GH, 1], F32, kind="ExternalInput").ap()
    wg2_d = nc.dram_tensor("wg2", [GH + 1, E], F32, kind="ExternalInput").ap()
    bg2_d = nc.dram_tensor("bg2", [1, E], F32, kind="ExternalInput").ap()
    bexp_d = nc.dram_tensor("b_exp", [E, 128], F32, kind="ExternalInput").ap()
    out_ap = nc.dram_tensor("out", [SPB, C, H, W], F16, kind="ExternalOutput").ap()

    with tile.TileContext(nc) as tc, ExitStack() as ctx:
        pools = {
            "const": ctx.enter_context(tc.tile_pool(name="const", bufs=1)),
            "xx": ctx.enter_context(tc.tile_pool(name="xx", bufs=SPB)),
            "gate": ctx.enter_context(tc.tile_pool(name="gate", bufs=2)),
            "wcomb": ctx.enter_context(tc.tile_pool(name="wcomb", bufs=2)),
            "stage": ctx.enter_context(tc.tile_pool(name="stage", bufs=6)),
            "scratch": ctx.enter_context(tc.tile_pool(name="scratch", bufs=1)),
            "gpsum": ctx.enter_context(tc.tile_pool(name="gpsum", bufs=1, space="PSUM")),
            "cpsum": ctx.enter_context(tc.tile_pool(name="cpsum", bufs=5, space="PSUM")),
        }
        cp = pools["const"]
        XX0 = pools["xx"].tile([128, FLAT], F16, tag="XX", name="XX0")
        XX1 = pools["xx"].tile([128, FLAT], F16, tag="XX", name="XX1")
        ones = cp.tile([1, 128], F32)
        nc.gpsimd.memset(ones[:], 1.0)
        # prewarm the ACT exp table before the ACT lane fills with DMAs
        warm = cp.tile([1, 1], F32)
        nc.scalar.activation(warm[:], ones[:, 0:1], ACTF.Exp)
        # tiny gate weights + expert weights B on the gpsimd SWDGE lane
        wg1x2_sb = cp.tile([128, GH], F32)
        nc.gpsimd.dma_start(wg1x2_sb[:], wg1_d[:])
        bg1_sb = cp.tile([GH, 1], F32)
        nc.gpsimd.dma_start(bg1_sb[:], bg1_d[:])
        wg2_sb = cp.tile([GH + 1, E], F32)
        nc.gpsimd.dma_start(wg2_sb[:], wg2_d[:])
        bg2_sb = cp.tile([1, E], F32)
        nc.gpsimd.dma_start(bg2_sb[:], bg2_d[:])
        bexp_sb = cp.tile([E, 128], F32)
        nc.gpsimd.dma_start(bexp_sb[:], bexp_d[:])
        wps_sb = cp.tile([128, E, 6, C], F16)
        nc.gpsimd.dma_start(wps_sb[:, E // 2 :], wpsB_d[:])

        def load_wpsA():
            nc.sync.dma_start(wps_sb[:, : E // 2], wpsA_d[:])

        pooled0, bots0 = _emit_sample_loads(nc, pools, 0, XX0, xs_ap, mid_sp=load_wpsA)
        consts = (wg1x2_sb, bg1_sb, wg2_sb, bg2_sb, bexp_sb, wps_sb, ones)

        g0 = _emit_sample_gate(nc, pools, 0, pooled0, consts)
        add_dep_helper(bots0[0].ins, g0[2].ins, sync=False,
                       reason="s0 late bottom DMAs after s0 softmax exp")
        pooled1, bots1 = _emit_sample_loads(nc, pools, 1, XX1, xs_ap)

        _emit_conv_pairs(nc, pools, 0, XX0, *g0[:2], out_ap, range(0, GATE_SPLIT))
        g1 = _emit_sample_gate(nc, pools, 1, pooled1, consts)
        _emit_conv_pairs(nc, pools, 0, XX0, *g0[:2], out_ap, range(GATE_SPLIT, NP))
        _emit_conv_pairs(nc, pools, 1, XX1, *g1[:2], out_ap, range(0, NP))

    nc.compile()
    _cache["nc"] = nc
    return nc


def host_prep(x, wg1, bg1, wg2, bg2, w_exp, b_exp):
    """Host-side layout prep + per-core sharding. Returns in_maps list."""
    x = np.asarray(x, dtype=np.float32).astype(np.float16)
    xpad = np.zeros((B, C, HP, WP), dtype=np.float16)
    xpad[:, :, 1:129, 1:129] = x
    xpad = xpad.reshape(B, C, FLAT)
    wg1 = np.asarray(wg1, dtype=np.float32)
    bg1 = np.asarray(bg1, dtype=np.float32).reshape(GH, 1)
    wg2 = np.asarray(wg2, dtype=np.float32)
    bg2 = np.asarray(bg2, dtype=np.float32).reshape(1, E)
    w_exp = np.asarray(w_exp, dtype=np.float32)
    b_exp = np.asarray(b_exp, dtype=np.float32)

    # w_exp [E, O, I, KH, KW] -> wt [I, E, KH, KW, O]
    wt = np.transpose(w_exp, (2, 0, 3, 4, 1)).copy()
    # residual shortcut: out += x == each expert's center tap += I
    # (routing probs sum to 1 up to the reference's 1e-8 epsilon)
    wt[:, :, 1, 1, :] += np.eye(C, dtype=np.float32)[:, None, :]
    # paired taps: top partitions = dx=-1, bottom = dx=+1
    wpair = np.concatenate([wt[:, :, :, 0, :], wt[:, :, :, 2, :]], axis=0)
    # single taps: dx=0 on top, zeros on bottom
    wsing = np.concatenate([wt[:, :, :, 1, :], np.zeros_like(wt[:, :, :, 1, :])], axis=0)
    # merged [128, E, 6, O]: slots 0-2 pairs, 3-5 singles
    wps = np.concatenate([wpair, wsing], axis=2).astype(np.float16)

    shared = {
        "wpsA": np.ascontiguousarray(wps[:, 0:4]),
        "wpsB": np.ascontiguousarray(wps[:, 4:8]),
        "wg1": np.ascontiguousarray(np.concatenate([wg1, wg1], axis=0) / (H * W)),
        "bg1": np.ascontiguousarray(bg1),
        "wg2": np.ascontiguousarray(np.concatenate([wg2, bg2], axis=0)),
        "bg2": np.ascontiguousarray(bg2),
        "b_exp": np.ascontiguousarray(np.concatenate([b_exp, b_exp], axis=1)),
    }
    return [
        {"xs": np.ascontiguousarray(xpad[SPB * k : SPB * (k + 1)]), **shared}
        for k in range(NCORES)
    ]


def kernel(x, wg1, bg1, wg2, bg2, w_exp, b_exp):
    nc = build_program()
    in_maps = host_prep(x, wg1, bg1, wg2, bg2, w_exp, b_exp)
    res = run_bass_kernel_spmd(nc, in_maps, list(range(NCORES)))
    return np.concatenate(
        [res.results[k]["out"].astype(np.float32) for k in range(NCORES)], axis=0
    )
